# revision 28
# baseline (speedup 1.0000x reference)
"""Trainium2 Bass kernel for nn_Attention_30760555774660 (stacked attention VQA).

Sharding: data-parallel over batch, 256 -> 8 cores x 32. Weights replicated.

Per-core structure (B=32, S=196, D=1024, A=512, O=3000), all matmuls bf16:
  - img is shipped bf16 and transposed during DRAM->SBUF load by the DMA
    XBAR (transpose=True), giving imgT [128d, 8c, 3136s] per 16-batch half.
    No PE transposes / PSUM copies for img at all.
  - Projection img @ W_ia runs s-flat: 25 chunks of [<=128 s, 512 a] PSUM,
    8 K-chunk matmuls each with imgT stationary and W_ia moving, plus a
    one-hot fold matmul adding the per-batch q-projection row.
  - tanh on ScalarE (psum -> bf16 SBUF); logits via one DVE
    tensor_tensor_reduce against a partition-broadcast Wp.
  - Logit columns [128, 25] are PE-transposed then reshaped to [16, 196]
    by a single SBUF->SBUF DMA; softmax is a 3-op sequence on 16 lanes.
  - vI via diag-masked piT stationaries accumulating all 16 batches into
    one PSUM [16, 512] pair; img natural layout is re-streamed from DRAM.
  - u1/u2 transposed once into u1T/u2T [128d, 32b] bf16 for the q-proj of
    block 2 and the final FC.
  - The two 16-batch halves are interleaved so softmax/DVE phases of one
    half hide under the other half's projection matmuls.
"""

import os
import sys

import numpy as np

if "/opt/trn_rl_repo" not in sys.path:
    sys.path.insert(0, "/opt/trn_rl_repo")

B_FULL = 256
N_CORES = 8
B = B_FULL // N_CORES  # 32
BH = 16  # half-batch
S = 196
D = 1024
A = 512
O = 3000
SH = BH * S  # 3136 flat s-cols per half
DC = D // 128  # 8
OC = 6
ON = O // OC  # 500
# flat s-chunks per half: 24 x 128 + 1 x 64
CHUNKS = [(j * 128, 128) for j in range(24)] + [(3072, 64)]
# xpose-DMA windows (s-cols) per half; first two small/early so proj can start
WINDOWS = [(0, 256), (256, 768), (1024, 1024), (2048, 1024), (3072, 64)]
EARLY_W = [0, 1]  # cover chunks 0..7
LATE_W = [2, 3, 4]

_nc_cache = None


def _build_nc():
    import concourse.bacc as bacc
    import concourse.tile as tile
    from concourse import mybir
    import bass_rust  # noqa: F401
    import concourse.bass as bass

    f32 = mybir.dt.float32
    bf16 = mybir.dt.bfloat16
    Tanh = mybir.ActivationFunctionType.Tanh
    Exp = mybir.ActivationFunctionType.Exp
    mult = mybir.AluOpType.mult
    add = mybir.AluOpType.add

    nc = bacc.Bacc("TRN2", target_bir_lowering=False)

    img_h = nc.dram_tensor("img", [B, S, D], bf16, kind="ExternalInput")
    ques_h = nc.dram_tensor("ques", [B, D], f32, kind="ExternalInput")
    wia1_h = nc.dram_tensor("W_ia1", [D, A], bf16, kind="ExternalInput")
    qp1_h = nc.dram_tensor("QP1H", [B, A], bf16, kind="ExternalInput")
    wp1_h = nc.dram_tensor("Wp1", [A], bf16, kind="ExternalInput")
    wia2_h = nc.dram_tensor("W_ia2", [D, A], bf16, kind="ExternalInput")
    wqa2_h = nc.dram_tensor("W_qa2", [D, A], bf16, kind="ExternalInput")
    bqa2_h = nc.dram_tensor("b_qa2", [A], f32, kind="ExternalInput")
    wp2_h = nc.dram_tensor("Wp2", [A], bf16, kind="ExternalInput")
    wfc_h = nc.dram_tensor("W_fc", [D, O], bf16, kind="ExternalInput")
    bfc_h = nc.dram_tensor("b_fc", [O], f32, kind="ExternalInput")
    sel_h = nc.dram_tensor("SEL", [BH, SH], bf16, kind="ExternalInput")
    identf_h = nc.dram_tensor("IDENTF", [128, 128], f32, kind="ExternalInput")
    score_h = nc.dram_tensor("score", [B, O], f32, kind="ExternalOutput")
    lcscr_h = nc.dram_tensor("LCSCR", [4, 3200], f32, kind="Internal")
    uscr_h = nc.dram_tensor("USCR", [4, BH, D], bf16, kind="Internal")

    def bcast_ap(h, n_part, off=0, n=None):
        ap = h[off : off + n] if n is not None else h[:]
        return bass.AP(tensor=ap.tensor, offset=ap.offset, ap=[[0, n_part]] + ap.ap)

    def diag_ap(t_ap, npart, nb):
        # t_ap: tile AP [128, nb, nb]; view [npart, nb] hitting [p, i, i]
        pstride = t_ap.ap[0][0]
        return bass.AP(
            tensor=t_ap.tensor, offset=t_ap.offset, ap=[[pstride, npart], [nb + 1, nb]]
        )

    with tile.TileContext(nc) as tc:
        with (
            tc.tile_pool(name="const", bufs=1) as const,
            tc.tile_pool(name="imgt", bufs=1) as imgt_p,
            tc.tile_pool(name="imgt0", bufs=2) as imgt0_p,
            tc.tile_pool(name="imgn", bufs=6) as imgn_p,
            tc.tile_pool(name="wst", bufs=2) as wst,
            tc.tile_pool(name="ha", bufs=3) as ha_p,
            tc.tile_pool(name="lc", bufs=2) as lc_p,
            tc.tile_pool(name="work", bufs=2) as work,
            tc.tile_pool(name="uh", bufs=1) as uh_p,
            tc.tile_pool(name="psp", bufs=5, space="PSUM") as psp,
            tc.tile_pool(name="pst", bufs=2, space="PSUM") as pst,
        ):
            # ---------------- constants / prologue ----------------
            identf = const.tile([128, 128], f32)
            nc.sync.dma_start(out=identf, in_=identf_h[:, :])
            sel = const.tile([BH, SH], bf16)
            nc.gpsimd.dma_start(out=sel, in_=sel_h[:, :])
            wia1 = const.tile([128, DC, A], bf16)
            nc.gpsimd.dma_start(out=wia1, in_=wia1_h[:, :].rearrange("(c p) a -> p c a", p=128))
            wia2 = const.tile([128, DC, A], bf16)
            nc.gpsimd.dma_start(out=wia2, in_=wia2_h[:, :].rearrange("(c p) a -> p c a", p=128))
            wqa2 = const.tile([128, DC, A], bf16)
            nc.gpsimd.dma_start(out=wqa2, in_=wqa2_h[:, :].rearrange("(c p) a -> p c a", p=128))
            wp1b = const.tile([128, A], bf16)
            nc.gpsimd.dma_start(out=wp1b, in_=bcast_ap(wp1_h, 128))
            wp2b = const.tile([128, A], bf16)
            nc.gpsimd.dma_start(out=wp2b, in_=bcast_ap(wp2_h, 128))
            bqa2b = const.tile([BH, A], f32)
            nc.gpsimd.dma_start(out=bqa2b, in_=bcast_ap(bqa2_h, BH))

            quesA = {}
            for h in range(2):
                qa = const.tile([BH, D], f32, tag=f"quesA{h}")
                nc.sync.dma_start(out=qa, in_=ques_h[h * BH : (h + 1) * BH, :])
                quesA[h] = qa

            # masks for vI: [s-part, b, b] diag tiles, memset once
            mask0 = const.tile([128, BH, BH], bf16)
            nc.vector.memset(mask0[:, :, :].bitcast(f32), 0.0)
            mask1 = const.tile([128, BH, BH], bf16)
            nc.vector.memset(mask1[:, :, :].bitcast(f32), 0.0)
            masks = [mask0, mask1]

            u1T = const.tile([128, DC, B], bf16)
            u2T = const.tile([128, DC, B], bf16)

            # QP1 precomputed on host
            QP1 = {}
            for h in range(2):
                qp = const.tile([BH, A], bf16, tag=f"QP1{h}")
                nc.sync.dma_start(out=qp, in_=qp1_h[h * BH : (h + 1) * BH, :])
                QP1[h] = qp

            # ---------------- imgT via DMA-XBAR transpose ----------------
            imgT = {}

            def load_imgT(h, windows):
                flat = img_h[h * BH : (h + 1) * BH, :, :].rearrange("b s d -> (b s) d")
                for w in windows:
                    w0, wl = WINDOWS[w]
                    pool = imgt0_p if w in EARLY_W else imgt_p
                    t = pool.tile([128, DC, wl], bf16, tag=f"imgt_{w}")
                    imgT[(h, w)] = t
                    nc.sync.dma_start(out=t, in_=flat[w0 : w0 + wl, :], transpose=True)

            def imgT_chunk(h, s0, sl):
                for w, (w0, wl) in enumerate(WINDOWS):
                    if w0 <= s0 < w0 + wl:
                        return imgT[(h, w)], s0 - w0
                raise AssertionError(s0)

            # ---------------- phase helpers ----------------
            Lc = {}

            def proj(h, blk, lo=0, hi=len(CHUNKS)):
                """Projection chunks [lo,hi) for half h, block blk."""
                wia = wia1 if blk == 0 else wia2
                wpb = wp1b if blk == 0 else wp2b
                qp = QP1[h] if blk == 0 else QP2[h]
                if (h, blk) not in Lc:
                    lc = lc_p.tile([128, len(CHUNKS)], f32, tag=f"lc{h}{blk}")
                    Lc[(h, blk)] = lc
                lc = Lc[(h, blk)]
                for j in range(lo, hi):
                    s0, sl = CHUNKS[j]
                    t, off = imgT_chunk(h, s0, sl)
                    pp = psp.tile([128, A], f32, tag="pp")
                    for c in range(DC):
                        nc.tensor.matmul(
                            pp[0:sl, :],
                            t[:, c, off : off + sl],
                            wia[:, c, :],
                            start=(c == 0),
                            stop=False,
                        )
                    nc.tensor.matmul(
                        pp[0:sl, :], sel[:, s0 : s0 + sl], qp, start=False, stop=True
                    )
                    ha = ha_p.tile([128, A], bf16, tag="ha")
                    nc.scalar.activation(ha[0:sl], pp[0:sl], Tanh)
                    tt = ha_p.tile([128, A], bf16, tag="tt")
                    nc.vector.tensor_mul(tt[0:sl], ha[0:sl], wpb[0:sl])
                    nc.vector.tensor_reduce(
                        lc[0:sl, j : j + 1],
                        tt[0:sl],
                        axis=mybir.AxisListType.X,
                        op=add,
                    )

            PIs = {}

            def softmax_pi(h, blk):
                """Lc[(h,blk)] -> PI [16, 196] fp32."""
                lc = Lc[(h, blk)]
                pt = pst.tile([len(CHUNKS), 128], f32, tag="tr")
                nc.tensor.transpose(pt, lc, identf)
                lcT = work.tile([len(CHUNKS), 128], f32, tag="lcT")
                nc.vector.tensor_copy(lcT, pt)
                idx = h * 2 + blk
                nc.scalar.dma_start(out=lcscr_h[idx, 0:3200], in_=lcT[:, :])
                LT = work.tile([BH, S], f32, tag="LT")
                nc.scalar.dma_start(
                    out=LT,
                    in_=lcscr_h[idx, 0:SH].rearrange("(b s) -> b s", b=BH),
                )
                E = work.tile([BH, S], f32, tag="E")
                Z = work.tile([BH, 1], f32, tag="Z")
                nc.scalar.activation(E, LT, Exp, accum_out=Z)
                R = work.tile([BH, 1], f32, tag="R")
                nc.vector.reciprocal(R, Z)
                PI = work.tile([BH, S], f32, tag="PI")
                nc.vector.tensor_scalar_mul(PI, E, R)
                PIs[(h, blk)] = PI

            def make_masks(h, blk):
                PI = PIs[(h, blk)]
                pa = pst.tile([128, BH], f32, tag="tr")
                nc.tensor.transpose(pa, PI[:, 0:128], identf[0:BH, 0:BH])
                nc.vector.tensor_copy(diag_ap(masks[0][:, :, :], 128, BH), pa)
                pb = pst.tile([128, BH], f32, tag="tr")
                nc.tensor.transpose(pb[0:68, :], PI[:, 128:196], identf[0:BH, 0:BH])
                nc.vector.tensor_copy(diag_ap(masks[1][:, :, :], 68, BH), pb[0:68, :])

            def vI_u(h, blk):
                """vI psum pair; u = vI + (ques | u1); returns u tile [16, 1024] f32."""
                vp0 = psp.tile([BH, A], f32, tag="pp")
                vp1 = psp.tile([BH, A], f32, tag="pp")
                vps = [vp0, vp1]
                k = 0
                for b in range(BH):
                    inb = imgN[(h, blk, b)]
                    for si, (p0, pl) in enumerate(((0, 128), (0, 68))):
                        for dh in range(2):
                            nc.tensor.matmul(
                                vps[dh],
                                masks[si][0:pl, b, :],
                                inb[0:pl, si, dh * A : (dh + 1) * A],
                                start=(k == 0),
                                stop=(k == 2 * BH - 1),
                            )
                        k += 1
                if blk == 0:
                    u = uh_p.tile([BH, D], f32, tag=f"u1_{h}")
                    other = quesA[h]
                else:
                    u = U1[h]
                    other = u
                for dh in range(2):
                    nc.vector.tensor_add(
                        u[:, dh * A : (dh + 1) * A], vps[dh], other[:, dh * A : (dh + 1) * A]
                    )
                return u

            def u_transpose(u, uT, h, idx):
                ub = work.tile([BH, D], bf16, tag="ub")
                nc.vector.tensor_copy(ub, u)
                nc.sync.dma_start(out=uscr_h[idx, :, :], in_=ub[:, :])
                nc.sync.dma_start(
                    out=uT[:, :, h * BH : (h + 1) * BH],
                    in_=uscr_h[idx, :, :],
                    transpose=True,
                )

            def qp2(h):
                qp_ps = psp.tile([BH, A], f32, tag="pp")
                for c in range(DC):
                    nc.tensor.matmul(
                        qp_ps,
                        u1T[:, c, h * BH : (h + 1) * BH],
                        wqa2[:, c, :],
                        start=(c == 0),
                        stop=(c == DC - 1),
                    )
                qp = const.tile([BH, A], bf16, tag=f"QP2{h}")
                nc.vector.tensor_add(qp, qp_ps, bqa2b)
                QP2[h] = qp

            def load_imgN(h, blk):
                for b in range(BH):
                    gb = h * BH + b
                    inb = imgn_p.tile([128, 2, D], bf16, tag="imgn")
                    imgN[(h, blk, b)] = inb
                    nc.gpsimd.dma_start(
                        out=inb[:, 0, :],
                        in_=img_h[gb : gb + 1, 0:128, :].rearrange("o s d -> (o s) d"),
                    )
                    nc.gpsimd.dma_start(
                        out=inb[0:68, 1, :],
                        in_=img_h[gb : gb + 1, 128:196, :].rearrange("o s d -> (o s) d"),
                    )

            def fc():
                for n in range(OC):
                    bfS = work.tile([BH, ON], f32, tag="bfS")
                    nc.gpsimd.dma_start(out=bfS, in_=bcast_ap(bfc_h, BH, n * ON, ON))
                    wf = wst.tile([128, DC, ON], bf16, tag="wf")
                    nc.sync.dma_start(
                        out=wf,
                        in_=wfc_h[:, :].rearrange("(c p) o -> p c o", p=128)[
                            :, :, n * ON : (n + 1) * ON
                        ],
                    )
                    for h in range(2):
                        fp = psp.tile([BH, ON], f32, tag="pp")
                        for c in range(DC):
                            nc.tensor.matmul(
                                fp,
                                u2T[:, c, h * BH : (h + 1) * BH],
                                wf[:, c, :],
                                start=(c == 0),
                                stop=(c == DC - 1),
                            )
                        sc = work.tile([BH, ON], f32, tag="sc")
                        nc.vector.tensor_add(sc, fp, bfS)
                        nc.sync.dma_start(
                            out=score_h[h * BH : (h + 1) * BH, n * ON : (n + 1) * ON],
                            in_=sc,
                        )

            # -------- main schedule: halves interleaved to hide softmax --------
            imgN = {}
            U1 = {}
            QP2 = {}

            load_imgT(0, EARLY_W + LATE_W)
            load_imgN(0, 0)
            proj(0, 0)

            load_imgT(1, EARLY_W)
            softmax_pi(0, 0)
            make_masks(0, 0)
            proj(1, 0, 0, 8)  # hides softmax(0,0); only needs h1 window 0

            U1[0] = vI_u(0, 0)
            u_transpose(U1[0], u1T, 0, 0)
            qp2(0)
            load_imgN(0, 1)
            proj(0, 1)

            load_imgT(1, LATE_W)
            proj(1, 0, 8, len(CHUNKS))  # hides softmax(0,1)
            softmax_pi(1, 0)
            softmax_pi(0, 1)
            make_masks(0, 1)

            u2_0 = vI_u(0, 1)
            u_transpose(u2_0, u2T, 0, 2)
            make_masks(1, 0)
            load_imgN(1, 0)

            U1[1] = vI_u(1, 0)
            u_transpose(U1[1], u1T, 1, 1)
            qp2(1)
            load_imgN(1, 1)
            proj(1, 1)

            softmax_pi(1, 1)
            make_masks(1, 1)

            u2_1 = vI_u(1, 1)
            u_transpose(u2_1, u2T, 1, 3)
            fc()

    nc.compile()
    return nc


def _get_nc():
    global _nc_cache
    if _nc_cache is None:
        _nc_cache = _build_nc()
    return _nc_cache


def _to_bf16(x):
    import ml_dtypes

    x = np.asarray(x)
    if x.nbytes >= 1 << 22:
        # big tensors: multithreaded conversion via jax CPU
        import jax

        cpu = jax.devices("cpu")[0]
        with jax.default_device(cpu):
            y = jax.jit(
                lambda v: v.astype("bfloat16"), backend="cpu"
            )(x)
            return np.asarray(y)
    return x.astype(ml_dtypes.bfloat16)


def _make_in_maps(inputs):
    import ml_dtypes

    bf = ml_dtypes.bfloat16
    ident = np.eye(128)
    selmat = np.zeros((BH, SH), np.float32)
    for b in range(BH):
        selmat[b, b * S : (b + 1) * S] = 1.0
    img_bf = _to_bf16(inputs["img_feat"])
    qp1_full = (
        np.asarray(inputs["ques_feat"], np.float32).astype(ml_dtypes.bfloat16).astype(np.float32)
        @ np.asarray(inputs["W_qa1"], np.float32).astype(ml_dtypes.bfloat16).astype(np.float32)
        + np.asarray(inputs["b_qa1"], np.float32)
    ).astype(bf)
    shared = {
        "W_ia1": _to_bf16(inputs["W_ia1"]),
        "Wp1": _to_bf16(inputs["Wp1"]),
        "W_ia2": _to_bf16(inputs["W_ia2"]),
        "W_qa2": _to_bf16(inputs["W_qa2"]),
        "b_qa2": np.ascontiguousarray(inputs["b_qa2"], np.float32),
        "Wp2": _to_bf16(inputs["Wp2"]),
        "W_fc": _to_bf16(inputs["W_fc"]),
        "b_fc": np.ascontiguousarray(inputs["b_fc"], np.float32),
        "SEL": selmat.astype(bf),
        "IDENTF": ident.astype(np.float32),
    }
    in_maps = []
    for c in range(N_CORES):
        sl = slice(c * B, (c + 1) * B)
        m = dict(shared)
        m["img"] = img_bf[sl]
        m["ques"] = np.ascontiguousarray(inputs["ques_feat"][sl], np.float32)
        m["QP1H"] = qp1_full[sl]
        in_maps.append(m)
    return in_maps


def kernel_run(inputs, trace=False):
    from concourse.bass_utils import run_bass_kernel_spmd

    nc = _get_nc()
    in_maps = _make_in_maps(inputs)
    res = run_bass_kernel_spmd(nc, in_maps, core_ids=list(range(N_CORES)), trace=trace)
    out = np.concatenate([r["score"] for r in res.results], axis=0)
    return out, res


def kernel(**inputs):
    out, _ = kernel_run(inputs)
    return out


# revision 29
# speedup vs baseline: 1.0330x; 1.0330x over previous
"""Trainium2 Bass kernel for nn_Attention_30760555774660 (stacked attention VQA).

Sharding: data-parallel over batch, 256 -> 8 cores x 32. Weights replicated.

Per-core structure (B=32, S=196, D=1024, A=512, O=3000), all matmuls bf16:
  - img is shipped bf16 and transposed during DRAM->SBUF load by the DMA
    XBAR (transpose=True), giving imgT [128d, 8c, 3136s] per 16-batch half.
    No PE transposes / PSUM copies for img at all.
  - Projection img @ W_ia runs s-flat: 25 chunks of [<=128 s, 512 a] PSUM,
    8 K-chunk matmuls each with imgT stationary and W_ia moving, plus a
    one-hot fold matmul adding the per-batch q-projection row.
  - tanh on ScalarE (psum -> bf16 SBUF); logits via one DVE
    tensor_tensor_reduce against a partition-broadcast Wp.
  - Logit columns [128, 25] are PE-transposed then reshaped to [16, 196]
    by a single SBUF->SBUF DMA; softmax is a 3-op sequence on 16 lanes.
  - vI via diag-masked piT stationaries accumulating all 16 batches into
    one PSUM [16, 512] pair; img natural layout is re-streamed from DRAM.
  - u1/u2 transposed once into u1T/u2T [128d, 32b] bf16 for the q-proj of
    block 2 and the final FC.
  - The two 16-batch halves are interleaved so softmax/DVE phases of one
    half hide under the other half's projection matmuls.
"""

import os
import sys

import numpy as np

if "/opt/trn_rl_repo" not in sys.path:
    sys.path.insert(0, "/opt/trn_rl_repo")

B_FULL = 256
N_CORES = 8
B = B_FULL // N_CORES  # 32
BH = 16  # half-batch
S = 196
D = 1024
A = 512
O = 3000
SH = BH * S  # 3136 flat s-cols per half
DC = D // 128  # 8
OC = 6
ON = O // OC  # 500
# flat s-chunks per half: 24 x 128 + 1 x 64
CHUNKS = [(j * 128, 128) for j in range(24)] + [(3072, 64)]
# xpose-DMA windows (s-cols) per half; first two small/early so proj can start
WINDOWS = [(0, 256), (256, 768), (1024, 1024), (2048, 1024), (3072, 64)]
EARLY_W = [0, 1]  # cover chunks 0..7
LATE_W = [2, 3, 4]

_nc_cache = None


def _build_nc():
    import concourse.bacc as bacc
    import concourse.tile as tile
    from concourse import mybir
    import bass_rust  # noqa: F401
    import concourse.bass as bass

    f32 = mybir.dt.float32
    bf16 = mybir.dt.bfloat16
    Tanh = mybir.ActivationFunctionType.Tanh
    Exp = mybir.ActivationFunctionType.Exp
    mult = mybir.AluOpType.mult
    add = mybir.AluOpType.add

    nc = bacc.Bacc("TRN2", target_bir_lowering=False)

    img_h = nc.dram_tensor("img", [B, S, D], bf16, kind="ExternalInput")
    ques_h = nc.dram_tensor("ques", [B, D], f32, kind="ExternalInput")
    wia1_h = nc.dram_tensor("W_ia1", [D, A], bf16, kind="ExternalInput")
    qp1_h = nc.dram_tensor("QP1H", [B, A], bf16, kind="ExternalInput")
    wp1_h = nc.dram_tensor("Wp1", [A], bf16, kind="ExternalInput")
    wia2_h = nc.dram_tensor("W_ia2", [D, A], bf16, kind="ExternalInput")
    wqa2_h = nc.dram_tensor("W_qa2", [D, A], bf16, kind="ExternalInput")
    bqa2_h = nc.dram_tensor("b_qa2", [A], f32, kind="ExternalInput")
    wp2_h = nc.dram_tensor("Wp2", [A], bf16, kind="ExternalInput")
    wfc_h = nc.dram_tensor("W_fc", [D, O], bf16, kind="ExternalInput")
    bfc_h = nc.dram_tensor("b_fc", [O], f32, kind="ExternalInput")
    sel_h = nc.dram_tensor("SEL", [BH, SH], bf16, kind="ExternalInput")
    identf_h = nc.dram_tensor("IDENTF", [128, 128], f32, kind="ExternalInput")
    score_h = nc.dram_tensor("score", [B, O], f32, kind="ExternalOutput")
    lcscr_h = nc.dram_tensor("LCSCR", [4, 3200], f32, kind="Internal")
    uscr_h = nc.dram_tensor("USCR", [4, BH, D], bf16, kind="Internal")

    def bcast_ap(h, n_part, off=0, n=None):
        ap = h[off : off + n] if n is not None else h[:]
        return bass.AP(tensor=ap.tensor, offset=ap.offset, ap=[[0, n_part]] + ap.ap)

    def diag_ap(t_ap, npart, nb):
        # t_ap: tile AP [128, nb, nb]; view [npart, nb] hitting [p, i, i]
        pstride = t_ap.ap[0][0]
        return bass.AP(
            tensor=t_ap.tensor, offset=t_ap.offset, ap=[[pstride, npart], [nb + 1, nb]]
        )

    with tile.TileContext(nc) as tc:
        with (
            tc.tile_pool(name="const", bufs=1) as const,
            tc.tile_pool(name="imgt", bufs=1) as imgt_p,
            tc.tile_pool(name="imgt0", bufs=2) as imgt0_p,
            tc.tile_pool(name="imgn", bufs=6) as imgn_p,
            tc.tile_pool(name="wst", bufs=2) as wst,
            tc.tile_pool(name="ha", bufs=3) as ha_p,
            tc.tile_pool(name="lc", bufs=2) as lc_p,
            tc.tile_pool(name="work", bufs=2) as work,
            tc.tile_pool(name="uh", bufs=1) as uh_p,
            tc.tile_pool(name="psp", bufs=5, space="PSUM") as psp,
            tc.tile_pool(name="pst", bufs=2, space="PSUM") as pst,
        ):
            # ---------------- constants / prologue ----------------
            identf = const.tile([128, 128], f32)
            nc.sync.dma_start(out=identf, in_=identf_h[:, :])
            sel = const.tile([BH, SH], bf16)
            nc.gpsimd.dma_start(out=sel, in_=sel_h[:, :])
            wia1 = const.tile([128, DC, A], bf16)
            nc.gpsimd.dma_start(out=wia1, in_=wia1_h[:, :].rearrange("(c p) a -> p c a", p=128))
            wia2 = const.tile([128, DC, A], bf16)
            nc.gpsimd.dma_start(out=wia2, in_=wia2_h[:, :].rearrange("(c p) a -> p c a", p=128))
            wqa2 = const.tile([128, DC, A], bf16)
            nc.gpsimd.dma_start(out=wqa2, in_=wqa2_h[:, :].rearrange("(c p) a -> p c a", p=128))
            wp1b = const.tile([128, A], bf16)
            nc.gpsimd.dma_start(out=wp1b, in_=bcast_ap(wp1_h, 128))
            wp2b = const.tile([128, A], bf16)
            nc.gpsimd.dma_start(out=wp2b, in_=bcast_ap(wp2_h, 128))
            bqa2b = const.tile([BH, A], f32)
            nc.gpsimd.dma_start(out=bqa2b, in_=bcast_ap(bqa2_h, BH))

            quesA = {}
            for h in range(2):
                qa = const.tile([BH, D], f32, tag=f"quesA{h}")
                nc.sync.dma_start(out=qa, in_=ques_h[h * BH : (h + 1) * BH, :])
                quesA[h] = qa

            # masks for vI: [s-part, b, b] diag tiles, memset once
            mask0 = const.tile([128, BH, BH], bf16)
            nc.vector.memset(mask0[:, :, :].bitcast(f32), 0.0)
            mask1 = const.tile([128, BH, BH], bf16)
            nc.vector.memset(mask1[:, :, :].bitcast(f32), 0.0)
            masks = [mask0, mask1]

            u1T = const.tile([128, DC, B], bf16)
            u2T = const.tile([128, DC, B], bf16)

            # QP1 precomputed on host
            QP1 = {}
            for h in range(2):
                qp = const.tile([BH, A], bf16, tag=f"QP1{h}")
                nc.sync.dma_start(out=qp, in_=qp1_h[h * BH : (h + 1) * BH, :])
                QP1[h] = qp

            # ---------------- imgT via DMA-XBAR transpose ----------------
            imgT = {}

            def load_imgT(h, windows):
                flat = img_h[h * BH : (h + 1) * BH, :, :].rearrange("b s d -> (b s) d")
                for w in windows:
                    w0, wl = WINDOWS[w]
                    pool = imgt0_p if w in EARLY_W else imgt_p
                    t = pool.tile([128, DC, wl], bf16, tag=f"imgt_{w}")
                    imgT[(h, w)] = t
                    nc.sync.dma_start(out=t, in_=flat[w0 : w0 + wl, :], transpose=True)

            def imgT_chunk(h, s0, sl):
                for w, (w0, wl) in enumerate(WINDOWS):
                    if w0 <= s0 < w0 + wl:
                        return imgT[(h, w)], s0 - w0
                raise AssertionError(s0)

            # ---------------- phase helpers ----------------
            Lc = {}

            def proj(h, blk, lo=0, hi=len(CHUNKS)):
                """Projection chunks [lo,hi) for half h, block blk."""
                wia = wia1 if blk == 0 else wia2
                wpb = wp1b if blk == 0 else wp2b
                qp = QP1[h] if blk == 0 else QP2[h]
                if (h, blk) not in Lc:
                    lc = lc_p.tile([128, len(CHUNKS)], f32, tag=f"lc{h}{blk}")
                    Lc[(h, blk)] = lc
                lc = Lc[(h, blk)]
                for j in range(lo, hi):
                    s0, sl = CHUNKS[j]
                    t, off = imgT_chunk(h, s0, sl)
                    pp = psp.tile([128, A], f32, tag="pp")
                    for c in range(DC):
                        nc.tensor.matmul(
                            pp[0:sl, :],
                            t[:, c, off : off + sl],
                            wia[:, c, :],
                            start=(c == 0),
                            stop=False,
                        )
                    nc.tensor.matmul(
                        pp[0:sl, :], sel[:, s0 : s0 + sl], qp, start=False, stop=True
                    )
                    ha = ha_p.tile([128, A], bf16, tag="ha")
                    nc.scalar.activation(ha[0:sl], pp[0:sl], Tanh)
                    tt = ha_p.tile([128, A], bf16, tag="tt")
                    nc.vector.tensor_mul(tt[0:sl], ha[0:sl], wpb[0:sl])
                    nc.vector.tensor_reduce(
                        lc[0:sl, j : j + 1],
                        tt[0:sl],
                        axis=mybir.AxisListType.X,
                        op=add,
                    )

            PIs = {}

            def softmax_pi(h, blk):
                """Lc[(h,blk)] -> PI [16, 196] fp32."""
                lc = Lc[(h, blk)]
                pt = pst.tile([len(CHUNKS), 128], f32, tag="tr")
                nc.tensor.transpose(pt, lc, identf)
                lcT = work.tile([len(CHUNKS), 128], f32, tag="lcT")
                nc.vector.tensor_copy(lcT, pt)
                idx = h * 2 + blk
                nc.scalar.dma_start(out=lcscr_h[idx, 0:3200], in_=lcT[:, :])
                LT = work.tile([BH, S], f32, tag="LT")
                nc.scalar.dma_start(
                    out=LT,
                    in_=lcscr_h[idx, 0:SH].rearrange("(b s) -> b s", b=BH),
                )
                E = work.tile([BH, S], f32, tag="E")
                Z = work.tile([BH, 1], f32, tag="Z")
                nc.scalar.activation(E, LT, Exp, accum_out=Z)
                R = work.tile([BH, 1], f32, tag="R")
                nc.vector.reciprocal(R, Z)
                PI = work.tile([BH, S], f32, tag="PI")
                nc.vector.tensor_scalar_mul(PI, E, R)
                PIs[(h, blk)] = PI

            def make_masks(h, blk):
                PI = PIs[(h, blk)]
                pa = pst.tile([128, BH], f32, tag="tr")
                nc.tensor.transpose(pa, PI[:, 0:128], identf[0:BH, 0:BH])
                nc.vector.tensor_copy(diag_ap(masks[0][:, :, :], 128, BH), pa)
                pb = pst.tile([128, BH], f32, tag="tr")
                nc.tensor.transpose(pb[0:68, :], PI[:, 128:196], identf[0:BH, 0:BH])
                nc.vector.tensor_copy(diag_ap(masks[1][:, :, :], 68, BH), pb[0:68, :])

            def vI_u(h, blk):
                """vI psum pair; u = vI + (ques | u1); returns u tile [16, 1024] f32."""
                vp0 = psp.tile([BH, A], f32, tag="pp")
                vp1 = psp.tile([BH, A], f32, tag="pp")
                vps = [vp0, vp1]
                k = 0
                for b in range(BH):
                    inb = imgN[(h, blk, b)]
                    for si, (p0, pl) in enumerate(((0, 128), (0, 68))):
                        for dh in range(2):
                            nc.tensor.matmul(
                                vps[dh],
                                masks[si][0:pl, b, :],
                                inb[0:pl, si, dh * A : (dh + 1) * A],
                                start=(k == 0),
                                stop=(k == 2 * BH - 1),
                            )
                        k += 1
                if blk == 0:
                    u = uh_p.tile([BH, D], f32, tag=f"u1_{h}")
                    other = quesA[h]
                else:
                    u = U1[h]
                    other = u
                for dh in range(2):
                    nc.vector.tensor_add(
                        u[:, dh * A : (dh + 1) * A], vps[dh], other[:, dh * A : (dh + 1) * A]
                    )
                return u

            def u_transpose(u, uT, h, idx):
                for c in range(DC):
                    pt = pst.tile([128, BH], f32, tag="tr")
                    nc.tensor.transpose(pt, u[:, c * 128 : (c + 1) * 128], identf[0:BH, 0:BH])
                    nc.vector.tensor_copy(uT[:, c, h * BH : (h + 1) * BH], pt)

            def qp2(h):
                qp_ps = psp.tile([BH, A], f32, tag="pp")
                for c in range(DC):
                    nc.tensor.matmul(
                        qp_ps,
                        u1T[:, c, h * BH : (h + 1) * BH],
                        wqa2[:, c, :],
                        start=(c == 0),
                        stop=(c == DC - 1),
                    )
                qp = const.tile([BH, A], bf16, tag=f"QP2{h}")
                nc.vector.tensor_add(qp, qp_ps, bqa2b)
                QP2[h] = qp

            def load_imgN(h, blk):
                for b in range(BH):
                    gb = h * BH + b
                    inb = imgn_p.tile([128, 2, D], bf16, tag="imgn")
                    imgN[(h, blk, b)] = inb
                    nc.gpsimd.dma_start(
                        out=inb[:, 0, :],
                        in_=img_h[gb : gb + 1, 0:128, :].rearrange("o s d -> (o s) d"),
                    )
                    nc.gpsimd.dma_start(
                        out=inb[0:68, 1, :],
                        in_=img_h[gb : gb + 1, 128:196, :].rearrange("o s d -> (o s) d"),
                    )

            def fc():
                for n in range(OC):
                    bfS = work.tile([BH, ON], f32, tag="bfS")
                    nc.gpsimd.dma_start(out=bfS, in_=bcast_ap(bfc_h, BH, n * ON, ON))
                    wf = wst.tile([128, DC, ON], bf16, tag="wf")
                    nc.sync.dma_start(
                        out=wf,
                        in_=wfc_h[:, :].rearrange("(c p) o -> p c o", p=128)[
                            :, :, n * ON : (n + 1) * ON
                        ],
                    )
                    for h in range(2):
                        fp = psp.tile([BH, ON], f32, tag="pp")
                        for c in range(DC):
                            nc.tensor.matmul(
                                fp,
                                u2T[:, c, h * BH : (h + 1) * BH],
                                wf[:, c, :],
                                start=(c == 0),
                                stop=(c == DC - 1),
                            )
                        sc = work.tile([BH, ON], f32, tag="sc")
                        nc.vector.tensor_add(sc, fp, bfS)
                        nc.sync.dma_start(
                            out=score_h[h * BH : (h + 1) * BH, n * ON : (n + 1) * ON],
                            in_=sc,
                        )

            # -------- main schedule: halves interleaved to hide softmax --------
            imgN = {}
            U1 = {}
            QP2 = {}

            load_imgT(0, EARLY_W + LATE_W)
            load_imgN(0, 0)
            proj(0, 0)

            load_imgT(1, EARLY_W)
            softmax_pi(0, 0)
            make_masks(0, 0)
            proj(1, 0, 0, 8)  # hides softmax(0,0); only needs h1 window 0

            U1[0] = vI_u(0, 0)
            u_transpose(U1[0], u1T, 0, 0)
            qp2(0)
            load_imgN(0, 1)
            proj(0, 1)

            load_imgT(1, LATE_W)
            proj(1, 0, 8, len(CHUNKS))  # hides softmax(0,1)
            softmax_pi(1, 0)
            softmax_pi(0, 1)
            make_masks(0, 1)

            u2_0 = vI_u(0, 1)
            u_transpose(u2_0, u2T, 0, 2)
            make_masks(1, 0)
            load_imgN(1, 0)

            U1[1] = vI_u(1, 0)
            u_transpose(U1[1], u1T, 1, 1)
            qp2(1)
            load_imgN(1, 1)
            proj(1, 1)

            softmax_pi(1, 1)
            make_masks(1, 1)

            u2_1 = vI_u(1, 1)
            u_transpose(u2_1, u2T, 1, 3)
            fc()

    nc.compile()
    return nc


def _get_nc():
    global _nc_cache
    if _nc_cache is None:
        _nc_cache = _build_nc()
    return _nc_cache


def _to_bf16(x):
    import ml_dtypes

    x = np.asarray(x)
    if x.nbytes >= 1 << 22:
        # big tensors: multithreaded conversion via jax CPU
        import jax

        cpu = jax.devices("cpu")[0]
        with jax.default_device(cpu):
            y = jax.jit(
                lambda v: v.astype("bfloat16"), backend="cpu"
            )(x)
            return np.asarray(y)
    return x.astype(ml_dtypes.bfloat16)


def _make_in_maps(inputs):
    import ml_dtypes

    bf = ml_dtypes.bfloat16
    ident = np.eye(128)
    selmat = np.zeros((BH, SH), np.float32)
    for b in range(BH):
        selmat[b, b * S : (b + 1) * S] = 1.0
    img_bf = _to_bf16(inputs["img_feat"])
    qp1_full = (
        np.asarray(inputs["ques_feat"], np.float32).astype(ml_dtypes.bfloat16).astype(np.float32)
        @ np.asarray(inputs["W_qa1"], np.float32).astype(ml_dtypes.bfloat16).astype(np.float32)
        + np.asarray(inputs["b_qa1"], np.float32)
    ).astype(bf)
    shared = {
        "W_ia1": _to_bf16(inputs["W_ia1"]),
        "Wp1": _to_bf16(inputs["Wp1"]),
        "W_ia2": _to_bf16(inputs["W_ia2"]),
        "W_qa2": _to_bf16(inputs["W_qa2"]),
        "b_qa2": np.ascontiguousarray(inputs["b_qa2"], np.float32),
        "Wp2": _to_bf16(inputs["Wp2"]),
        "W_fc": _to_bf16(inputs["W_fc"]),
        "b_fc": np.ascontiguousarray(inputs["b_fc"], np.float32),
        "SEL": selmat.astype(bf),
        "IDENTF": ident.astype(np.float32),
    }
    in_maps = []
    for c in range(N_CORES):
        sl = slice(c * B, (c + 1) * B)
        m = dict(shared)
        m["img"] = img_bf[sl]
        m["ques"] = np.ascontiguousarray(inputs["ques_feat"][sl], np.float32)
        m["QP1H"] = qp1_full[sl]
        in_maps.append(m)
    return in_maps


def kernel_run(inputs, trace=False):
    from concourse.bass_utils import run_bass_kernel_spmd

    nc = _get_nc()
    in_maps = _make_in_maps(inputs)
    res = run_bass_kernel_spmd(nc, in_maps, core_ids=list(range(N_CORES)), trace=trace)
    out = np.concatenate([r["score"] for r in res.results], axis=0)
    return out, res


def kernel(**inputs):
    out, _ = kernel_run(inputs)
    return out
